# revision 25
# baseline (speedup 1.0000x reference)
"""PointGroup clusters_voxelization kernel for Trainium2 (8 NeuronCores).

Strategy: shard the 1024 clusters across 8 cores (128 each); feats/coords
replicated via a packed f32 table so each point is one 140B row.

The random per-point gather is the hard constraint: TRN2's indirect DMA
(SWDGE, Pool engine) consumes exactly ONE offset per SBUF partition per
instruction (the compiled descriptor expansion streams each partition's
whole destination extent from its first offset -- multi-offset batched
forms execute incorrectly on hardware).  That fixes the gather at 2048
instructions per core x ~1.04us of Pool-engine descriptor generation
each, making the Pool engine the sole bottleneck; every other stage is
structured to run entirely in its shadow:

  - 15 rounds of 8 clusters plus a geometric tail (4, 2, 2 clusters)
    so the last exposed stats->params->write chain is minimal; a cluster
    spans P/GC partitions, a round is SBUF-resident, and the output rows
    of a partition are contiguous in DRAM
  - per round: 128 one-row-per-partition gathers -> min/max coord stats
    (2 reduces with components as the middle AP dim) -> cross-partition
    regroup via a tiny SBUF->SBUF DMA -> per-cluster params on 8
    partitions -> broadcast -> full-row copy to a staging tile on the
    otherwise-idle Activation engine -> fused coord transform in place
    -> one large-descriptor contiguous write per round
  - the segment mean cancels algebraically (out = raw*s - min*s +
    t0*j0 + t1*j1), so only min/max are reduced and the params chain is
    short

Total: Pool ~2.13ms busy; DMA engines ~250us; DVE/Act well under that.
"""
import numpy as np

import concourse.bass as bass
import concourse.bacc as bacc
import concourse.tile as tile
import concourse.mybir as mybir
from concourse import bass_utils

N = 1048576
C = 32
NCLUSTER = 1024
PTS = 2048
S = NCLUSTER * PTS
NCORES = 8
P = 128                      # SBUF partitions
PPC = S // NCORES            # points per core = 262144
ROW = C + 3                  # 35 floats per row
RNDS = 16                    # rounds per core
GC = 8                       # clusters per round
QP = P // GC                 # partitions per cluster = 16
PP = PTS // QP               # points per partition per round = 128

_CACHE = {}


def _build_program(fullscale: float, scale: float):
    key = (fullscale, scale)
    if key in _CACHE:
        return _CACHE[key]

    fs = float(fullscale)
    sc = float(scale)
    f32 = mybir.dt.float32

    nc = bacc.Bacc("TRN2", target_bir_lowering=False, debug=False)
    table_d = nc.dram_tensor("table", (N, ROW), f32, kind="ExternalInput")
    pid_d = nc.dram_tensor("pid", (PPC,), mybir.dt.int32, kind="ExternalInput")
    jit_d = nc.dram_tensor("jit", (2, 3), f32, kind="ExternalInput")
    out_d = nc.dram_tensor("out", (PPC, ROW), f32, kind="ExternalOutput")

    with tile.TileContext(nc) as tc:
        with (
            tc.tile_pool(name="one", bufs=1) as one,
            tc.tile_pool(name="gat", bufs=4) as gat,
            tc.tile_pool(name="pck", bufs=2) as pck,
            tc.tile_pool(name="sm", bufs=3) as smp,
        ):
            # point ids, laid out so partition p of round r covers the PP
            # consecutive points starting at P*PP*r + PP*p.  Round 0's slab
            # loads first so the first gather isn't gated on the full load.
            idx_t = one.tile([P, RNDS * PP], mybir.dt.int32)
            nc.sync.dma_start(
                out=idx_t[:, 0:PP],
                in_=bass.AP(tensor=pid_d, offset=0, ap=[[PP, P], [1, PP]]),
            )
            nc.sync.dma_start(
                out=idx_t[:, PP : 15 * PP],
                in_=bass.AP(
                    tensor=pid_d, offset=P * PP,
                    ap=[[PP, P], [P * PP, 14], [1, PP]],
                ),
            )
            # the small tail rounds have different per-partition strides
            nc.sync.dma_start(
                out=idx_t[:, 15 * PP : 15 * PP + 64],
                in_=bass.AP(tensor=pid_d, offset=15 * P * PP,
                            ap=[[64, P], [1, 64]]),
            )
            nc.sync.dma_start(
                out=idx_t[:, 15 * PP + 64 :],
                in_=bass.AP(
                    tensor=pid_d, offset=15 * P * PP + P * 64,
                    ap=[[32, P], [P * 32, 2], [1, 32]],
                ),
            )
            jit_t = one.tile([P, 6], f32)
            jsrc = jit_d.ap().rearrange("a b -> (a b)")
            nc.sync.dma_start(
                out=jit_t[:],
                in_=bass.AP(tensor=jsrc.tensor, offset=jsrc.offset,
                            ap=[[0, P]] + jsrc.ap),
            )

            # 15 full rounds of 8 clusters, then two 4-cluster rounds so
            # the post-last-gather stats->params->write chain (the only part
            # not hidden under the Pool gather stream) is half-sized
            rounds = [(8, 16, 128)] * 15 + [(4, 32, 64)] + [(2, 64, 32)] * 2
            lane0 = 0
            for r, (gc, qp, pp) in enumerate(rounds):
                l0 = lane0
                lane0 += pp
                # --- gather: pp one-row-per-partition indirect DMAs ---
                asm = gat.tile([P, PP, ROW], f32, name="asm")
                for k in range(pp):
                    nc.gpsimd.indirect_dma_start(
                        out=asm[:, k, :],
                        out_offset=None,
                        in_=table_d.ap(),
                        in_offset=bass.IndirectOffsetOnAxis(
                            ap=idx_t[:, l0 + k : l0 + k + 1], axis=0
                        ),
                    )

                # --- per-partition coord stats: one min + one max reduce,
                # components as the middle AP dim (rows innermost) ---
                st = smp.tile([P, 8], f32, name="st")
                asm_ap = asm[:]
                coords_ap = bass.AP(tensor=asm_ap.tensor,
                                    offset=asm_ap.offset + C,
                                    ap=[asm_ap.ap[0], [1, 3], [ROW, pp]])
                nc.vector.tensor_reduce(
                    out=st[:, 0:3], in_=coords_ap,
                    axis=mybir.AxisListType.X, op=mybir.AluOpType.min,
                )
                nc.vector.tensor_reduce(
                    out=st[:, 3:6], in_=coords_ap,
                    axis=mybir.AxisListType.X, op=mybir.AluOpType.max,
                )

                # --- regroup: cluster c's QP partial stat rows land on
                # partition c as QP consecutive 6-lane blocks ---
                stg = smp.tile([GC, 6 * 64], f32, name="stg")
                nc.sync.dma_start(out=stg[0:gc, 0 : 6 * qp], in_=st[:, 0:6])
                # combine across the QP blocks: stat j is at lanes j, j+6, ...
                red = smp.tile([GC, 8], f32, name="red")
                stg_ap = stg[:]
                for i, op in enumerate(
                    (mybir.AluOpType.min, mybir.AluOpType.max)
                ):
                    nc.vector.tensor_reduce(
                        out=red[:, 3 * i : 3 * i + 3],
                        in_=bass.AP(tensor=stg_ap.tensor,
                                    offset=stg_ap.offset + 3 * i,
                                    ap=[stg_ap.ap[0], [1, 3], [6, qp]]),
                        axis=mybir.AxisListType.X, op=op,
                    )

                # --- per-cluster params on GC partitions, folded to
                # out = raw * s + b.  The segment mean cancels algebraically:
                #   out = (raw-cm)*s + (-(min-cm)*s + t0*j0 + t1*j1)
                #       = raw*s - min*s + t0*j0 + t1*j1
                # with wd = max-min, t = fs - wd*s, t0 = max(t-.001, 0),
                # t1 = min(t+.001, 0) -- so only min/max are needed.
                pr = smp.tile([GC, 16], f32, name="pr")
                WD, T0, T1, MS = (
                    slice(0, 3), slice(3, 6), slice(6, 9), slice(9, 12),
                )
                sc_t = smp.tile([GC, 4], f32, name="sc_t")
                nc.vector.tensor_tensor(
                    out=pr[:, WD], in0=red[:, 3:6], in1=red[:, 0:3],
                    op=mybir.AluOpType.subtract,
                )
                nc.vector.reduce_max(
                    out=sc_t[:, 0:1], in_=pr[:, WD], axis=mybir.AxisListType.X
                )
                # s = min(fs/wmax - 0.01, scale) via IEEE reciprocal
                nc.vector.reciprocal(out=sc_t[:, 1:2], in_=sc_t[:, 0:1])
                nc.vector.tensor_scalar(
                    out=sc_t[:, 2:3], in0=sc_t[:, 1:2], scalar1=fs, scalar2=-0.01,
                    op0=mybir.AluOpType.mult, op1=mybir.AluOpType.add,
                )
                nc.vector.tensor_scalar(
                    out=sc_t[:, 2:3], in0=sc_t[:, 2:3], scalar1=sc, scalar2=None,
                    op0=mybir.AluOpType.min,
                )
                s_ap = sc_t[:, 2:3]
                # t = fs - wd*s ; t0 = max(t-.001, 0) ; t1 = min(t+.001, 0)
                nc.vector.tensor_scalar(
                    out=pr[:, T0], in0=pr[:, WD], scalar1=s_ap, scalar2=None,
                    op0=mybir.AluOpType.mult,
                )
                nc.vector.tensor_scalar(
                    out=pr[:, T0], in0=pr[:, T0], scalar1=-1.0, scalar2=fs,
                    op0=mybir.AluOpType.mult, op1=mybir.AluOpType.add,
                )
                nc.vector.tensor_scalar(
                    out=pr[:, T1], in0=pr[:, T0], scalar1=0.001, scalar2=0.0,
                    op0=mybir.AluOpType.add, op1=mybir.AluOpType.min,
                )
                nc.vector.tensor_scalar(
                    out=pr[:, T0], in0=pr[:, T0], scalar1=-0.001, scalar2=0.0,
                    op0=mybir.AluOpType.add, op1=mybir.AluOpType.max,
                )
                # b = t0*j0 + t1*j1 - min*s
                nc.vector.tensor_tensor(
                    out=pr[:, T0], in0=pr[:, T0], in1=jit_t[0:GC, 0:3],
                    op=mybir.AluOpType.mult,
                )
                nc.vector.tensor_tensor(
                    out=pr[:, T1], in0=pr[:, T1], in1=jit_t[0:GC, 3:6],
                    op=mybir.AluOpType.mult,
                )
                nc.vector.tensor_scalar(
                    out=pr[:, MS], in0=red[:, 0:3], scalar1=s_ap, scalar2=None,
                    op0=mybir.AluOpType.mult,
                )
                prm = smp.tile([GC, 4], f32, name="prm")
                nc.vector.tensor_copy(out=prm[:, 0:1], in_=s_ap)
                nc.vector.tensor_tensor(
                    out=pr[:, T0], in0=pr[:, T0], in1=pr[:, T1],
                    op=mybir.AluOpType.add,
                )
                nc.vector.tensor_tensor(
                    out=prm[:, 1:4], in0=pr[:, T0], in1=pr[:, MS],
                    op=mybir.AluOpType.subtract,
                )

                # --- broadcast [s, b0, b1, b2] to the QP partitions of each
                # cluster via 0-stride re-read ---
                prmb = smp.tile([P, 4], f32, name="prmb")
                prm_ap = prm[:]
                nc.sync.dma_start(
                    out=prmb[:],
                    in_=bass.AP(tensor=prm_ap.tensor, offset=prm_ap.offset,
                                ap=[[prm_ap.ap[0][0], gc], [0, qp], [1, 4]]),
                )

                # --- stage rows (Activation engine; also the only late reader
                # of asm), then transform the coord lanes in place ---
                pk = pck.tile([P, PP, ROW], f32, name="pk")
                nc.scalar.copy(out=pk[:, 0:pp, :], in_=asm[:, 0:pp, :])
                for c in range(3):
                    nc.vector.tensor_scalar(
                        out=pk[:, 0:pp, C + c], in0=pk[:, 0:pp, C + c],
                        scalar1=prmb[:, 0:1], scalar2=prmb[:, 1 + c : 2 + c],
                        op0=mybir.AluOpType.mult, op1=mybir.AluOpType.add,
                    )

                # --- one large-descriptor write of the round's final rows ---
                nc.sync.dma_start(
                    out=bass.AP(tensor=out_d, offset=l0 * P * ROW,
                                ap=[[pp * ROW, P], [1, pp * ROW]]),
                    in_=pk[:, 0:pp, :],
                )

    nc.compile()
    _CACHE[key] = nc
    return nc


def _reference_numpy(clusters_idx, clusters_offset, feats, coords, jitter, fullscale, scale):
    seg = clusters_idx[:, 0].astype(np.int64)
    pid = clusters_idx[:, 1].astype(np.int64)
    nC = clusters_offset.shape[0] - 1
    fs = np.float32(fullscale)
    cf = feats[pid]
    cc = coords[pid].astype(np.float32)
    cnt = np.diff(clusters_offset).astype(np.float32)[:, None]
    sums = np.zeros((nC, 3), np.float32)
    np.add.at(sums, seg, cc)
    cmean = sums / np.maximum(cnt, 1.0)
    ccc = cc - cmean[seg]
    cmin = np.full((nC, 3), np.inf, np.float32)
    cmax = np.full((nC, 3), -np.inf, np.float32)
    np.minimum.at(cmin, seg, ccc)
    np.maximum.at(cmax, seg, ccc)
    cscale = 1.0 / ((cmax - cmin) / fs).max(axis=1) - np.float32(0.01)
    cscale = np.minimum(cscale, np.float32(scale)).astype(np.float32)
    mn = cmin * cscale[:, None]
    mx = cmax * cscale[:, None]
    ccc = ccc * cscale[seg][:, None]
    rng = mx - mn
    off = (-mn + np.maximum(fs - rng - 0.001, 0.0) * jitter[0]
           + np.minimum(fs - rng + 0.001, 0.0) * jitter[1]).astype(np.float32)
    ccc = ccc + off[seg]
    return np.concatenate([cf, ccc], axis=1).astype(np.float32)


def _make_in_maps(clusters_idx, feats, coords, jitter):
    table = np.ascontiguousarray(
        np.concatenate([feats, coords], axis=1), dtype=np.float32
    )
    pid_full = np.ascontiguousarray(clusters_idx[:, 1].astype(np.int32))
    in_maps = []
    for k in range(NCORES):
        in_maps.append(
            {
                "table": table,
                "pid": pid_full[k * PPC : (k + 1) * PPC],
                "jit": jitter,
            }
        )
    return in_maps


def kernel(clusters_idx, clusters_offset, feats, coords, jitter, fullscale, scale):
    clusters_idx = np.asarray(clusters_idx)
    clusters_offset = np.asarray(clusters_offset)
    feats = np.asarray(feats, dtype=np.float32)
    coords = np.asarray(coords, dtype=np.float32)
    jitter = np.asarray(jitter, dtype=np.float32)

    fs = float(np.asarray(fullscale).item()) if not isinstance(fullscale, (int, float)) else float(fullscale)
    sc = float(np.asarray(scale).item()) if not isinstance(scale, (int, float)) else float(scale)

    uniform = (
        clusters_idx.shape == (S, 2)
        and clusters_offset.shape == (NCLUSTER + 1,)
        and feats.shape == (N, C)
        and coords.shape == (N, 3)
        and np.array_equal(
            clusters_offset,
            np.arange(NCLUSTER + 1, dtype=np.int64) * PTS,
        )
        and np.array_equal(
            clusters_idx[:, 0],
            np.repeat(np.arange(NCLUSTER, dtype=np.int64), PTS),
        )
    )
    if not uniform:
        return _reference_numpy(
            clusters_idx, clusters_offset, feats, coords, jitter, fs, sc
        )

    nc = _build_program(fs, sc)
    in_maps = _make_in_maps(clusters_idx, feats, coords, jitter)
    res = bass_utils.run_bass_kernel_spmd(nc, in_maps, core_ids=list(range(NCORES)))
    return np.concatenate([res.results[k]["out"] for k in range(NCORES)], axis=0)
